# revision 9
# baseline (speedup 1.0000x reference)
"""Trainium2 Bass kernel for the masked MQA attention block (nn_Attention_4252017623134).

Sharding: pure data-parallel over batch. b=8 batch elements, 8 NeuronCores,
one batch element per core, weights replicated. No collectives.

Per-core math (n=1024, d=1024, h=16, dh=64, inner=1024):
  context = x                      (pre-norm residual branch feeds K/V)
  xn  = layernorm(x) * g_in        (g_in folded into Wq on host)
  q   = xn @ Wq.T                  (dh^-0.5 folded into exp scale)
  k,v = context @ Wkv.T            (single shared KV head) + null_kv token
  att = softmax(mask(q k^T / 8))   (padding + causal masks)
  out = layernorm(att @ v @ Wo.T) * g_out

v2 design notes (vs the f32r baseline):
  * bf16 everywhere on matmul moving operands (full PE rate at any N; cost
    model charges 1 cycle/moving-row regardless of K/M). Host pre-casts
    x/xT/weights to bf16 so no on-chip rounding copies are needed. Stationary
    operands that come from PSUM stay f32r (kT2, outT) for precision.
  * All transposes run on the DMA engines (dma_start_transpose, 16x128 xbar
    tiles) - the PE does zero transposes.
  * Null-token scores use a host-precomposed W_null = null_k @ (Wq*g_in)
    [16 x D], so they only need xnT, not qT.
  * Padding mask is applied by zeroing masked key rows of v_aug (numerator
    AND denominator drop those keys); exp needs no bias at all.
  * v_aug carries 64 replicated ones-columns, so the PV matmul leaves the
    softmax denominator broadcast across PSUM partitions 64:128; division is
    a reciprocal + one fused evacuate-multiply per (head, chunk).
  * Causal masking is an affine_select (fill=-30000) on the single diagonal
    128x128 PSUM band, on the gpsimd engine, before exp.
  * Engine budget: PE ~130us (kv 3.4 + q 27.3 + wnull 3.4 + scores 30.7 +
    PV 37.5 + out-proj 27.3), ACT ~85us (exp dominated), DVE ~65us,
    Pool ~55us, DMA ~45us.
"""

import contextlib

import numpy as np
import ml_dtypes

import concourse.bass as bass
import concourse.bacc as bacc
import concourse.tile as tile
import concourse.mybir as mybir
from concourse.bass_utils import run_bass_kernel_spmd

N = 1024          # sequence length per core
D = 1024          # model dim
H = 16            # query heads
DH = 64           # head dim
INNER = H * DH    # 1024
NT = N // 128     # 8 i-tiles / j-tiles / d-tiles
EPS = 1e-5
MASK_BIAS = -30000.0  # additive causal bias; exp(-3750) == 0.0 in fp32

F32 = mybir.dt.float32
F32R = mybir.dt.float32r
BF16 = mybir.dt.bfloat16
U8 = mybir.dt.uint8
AF = mybir.ActivationFunctionType
ALU = mybir.AluOpType


def _emit(nc):
    # ---------------- DRAM I/O ----------------
    x_d = nc.dram_tensor("x", [N, D], BF16, kind="ExternalInput")
    xT_d = nc.dram_tensor("xT", [D, N], BF16, kind="ExternalInput")
    wqT_d = nc.dram_tensor("wqT", [D, INNER], BF16, kind="ExternalInput")
    wnT_d = nc.dram_tensor("wnT", [D, H], BF16, kind="ExternalInput")
    wkvT_d = nc.dram_tensor("wkvT", [D, 2 * DH], BF16, kind="ExternalInput")
    woT_d = nc.dram_tensor("woT", [INNER, D], F32R, kind="ExternalInput")
    vnull_d = nc.dram_tensor("vnull", [1, DH], F32, kind="ExternalInput")
    mask_d = nc.dram_tensor("mask", [N], U8, kind="ExternalInput")
    gout_d = nc.dram_tensor("gout", [D], BF16, kind="ExternalInput")
    out_d = nc.dram_tensor("out", [N, D], BF16, kind="ExternalOutput")

    d_ = dict(x_d=x_d, xT_d=xT_d, wqT_d=wqT_d, wnT_d=wnT_d, wkvT_d=wkvT_d,
              woT_d=woT_d, vnull_d=vnull_d, mask_d=mask_d, gout_d=gout_d,
              out_d=out_d)
    with tile.TileContext(nc) as tc:
        _emit_tile(nc, tc, d_)
    return nc


def _emit_tile(nc, tc, d_):
    x_d, xT_d = d_["x_d"], d_["xT_d"]
    wqT_d, wnT_d, wkvT_d, woT_d = d_["wqT_d"], d_["wnT_d"], d_["wkvT_d"], d_["woT_d"]
    vnull_d, mask_d = d_["vnull_d"], d_["mask_d"]
    gout_d, out_d = d_["gout_d"], d_["out_d"]

    ctx = contextlib.ExitStack()
    with ctx:
        consts = ctx.enter_context(tc.tile_pool(name="consts", bufs=1))
        persist = ctx.enter_context(tc.tile_pool(name="persist", bufs=1))
        stage = ctx.enter_context(tc.tile_pool(name="stage", bufs=4))

        # ---------------- constants ----------------
        maskf = consts.tile([128, NT], F32)
        mask_u8 = consts.tile([128, NT], U8)
        nc.sync.dma_start(out=mask_u8[:],
                          in_=bass.AP(tensor=mask_d, offset=0,
                                      ap=[[1, 128], [128, NT]]))
        nc.vector.tensor_copy(maskf[:], mask_u8[:])
        gout_bb = consts.tile([128, D], BF16)
        nc.sync.dma_start(out=gout_bb[:],
                          in_=bass.AP(tensor=gout_d, offset=0,
                                      ap=[[0, 128], [1, D]]))
        eps_t = consts.tile([128, 1], F32)
        nc.vector.memset(eps_t[:], EPS)
        # warm the ACT function tables (Sqrt/Exp) so the first LN/softmax
        # doesn't pay the ~1.3us table load inside its dependency chain
        warm = consts.tile([128, 2], F32)
        nc.scalar.activation(out=warm[:, 0:1], in_=eps_t[:], func=AF.Sqrt)
        nc.scalar.activation(out=warm[:, 1:2], in_=eps_t[:], func=AF.Exp)

        # ------------- whole-kernel persistent tiles -------------
        kT2 = persist.tile([128, N], BF16, tag="kT2")        # both 64-halves = k^T
        v_aug = persist.tile([128, NT, 128], BF16, tag="v_aug")  # [:, t, 0:64]=v, 64:128=ones
        vnull_aug = persist.tile([1, 128], BF16, tag="vnull")    # [v_null | ones]
        outT = persist.tile([128, NT, N], F32R, tag="outT")  # attn out^T / denom-divided
        unh = persist.tile([1, H, N], BF16, tag="unh")       # per-head null-token exp rows

        # ============ Phases A+B window (qT lives across both) ============
        with tc.tile_pool(name="poolAB", bufs=1) as poolAB:
            qT = poolAB.tile([128, NT, N], BF16, tag="qT")
            xnT = poolAB.tile([128, NT, N], BF16, tag="xnT")

            # ---- Phase A: loads, LN1, projections ----
            with tc.tile_pool(name="poolA", bufs=1) as poolA:
                wqT_s = poolA.tile([128, NT, INNER], BF16, tag="wqT")
                wkvT_s = poolA.tile([128, NT, 2 * DH], BF16, tag="wkvT")
                wnT_s = poolA.tile([128, NT, H], BF16, tag="wnT")
                vT_s = poolA.tile([64, N], BF16, tag="vT_s")
                uall = poolA.tile([H, N], BF16, tag="uall")

                nc.sync.dma_start(
                    out=wkvT_s[:],
                    in_=wkvT_d.ap().rearrange("(t p) c -> p t c", p=128))
                vn_s = stage.tile([1, DH], F32, tag="nk2")
                nc.sync.dma_start(out=vn_s[:], in_=vnull_d.ap())

                # interleave x (natural, for LN1) and xT (for kv) loads
                x_tiles = []
                xT_tiles = []
                for it in range(NT):
                    xs = poolA.tile([128, D], BF16, tag=f"x{it}")
                    nc.sync.dma_start(out=xs[:], in_=x_d[it * 128:(it + 1) * 128, :])
                    x_tiles.append(xs)
                    xt = poolA.tile([128, N], BF16, tag=f"xT{it}")
                    nc.sync.dma_start(out=xt[:], in_=xT_d[it * 128:(it + 1) * 128, :])
                    xT_tiles.append(xt)
                nc.sync.dma_start(
                    out=wqT_s[:],
                    in_=wqT_d.ap().rearrange("(t p) c -> p t c", p=128))
                nc.sync.dma_start(
                    out=wnT_s[:],
                    in_=wnT_d.ap().rearrange("(t p) c -> p t c", p=128))

                # --- kv projection: kvT[c, j] accumulated over d-tiles ---
                with tc.tile_pool(name="psKV", bufs=1, space="PSUM") as psKV:
                    pkv = psKV.tile([128, N], F32, tag="pkv")
                    for t in range(NT):
                        for ch in range(2):
                            nc.tensor.matmul(pkv[:, ch * 512:(ch + 1) * 512],
                                             wkvT_s[:, t, :],
                                             xT_tiles[t][:, ch * 512:(ch + 1) * 512],
                                             start=(t == 0), stop=(t == NT - 1))
                    # evacuate: k rows 0:64 duplicated into both kT2 halves (ACT),
                    # v rows 64:128 -> vT_s bf16 (DVE)
                    nc.scalar.copy(kT2[0:64, :], pkv[0:64, :])
                    nc.scalar.copy(kT2[64:128, :], pkv[0:64, :])
                    nc.vector.tensor_copy(vT_s[:], pkv[64:128, :])

                # --- LN1 pipeline + DMA transpose into xnT ---
                for it in range(NT):
                    x_s = x_tiles[it]
                    st = stage.tile([128, 2, 6], F32, tag="bnst")
                    nc.vector.bn_stats(out=st[:, 0, :], in_=x_s[:, 0:512])
                    nc.vector.bn_stats(out=st[:, 1, :], in_=x_s[:, 512:1024])
                    mv = stage.tile([128, 2], F32, tag="bnmv")
                    nc.vector.bn_aggr(out=mv[:], in_=st[:])
                    rstd = stage.tile([128, 1], F32, tag="rstd")
                    nc.scalar.activation(out=rstd[:], in_=mv[:, 1:2], func=AF.Sqrt,
                                         bias=eps_t[:], scale=1.0)
                    nc.vector.reciprocal(out=rstd[:], in_=rstd[:])
                    xn_s = stage.tile([128, D], BF16, tag="xnft")
                    nc.vector.tensor_scalar(out=xn_s[:], in0=x_s[:],
                                            scalar1=mv[:, 0:1], scalar2=rstd[:],
                                            op0=ALU.subtract, op1=ALU.mult)
                    nc.sync.dma_start_transpose(
                        out=xnT[:, :, it * 128:(it + 1) * 128], in_=xn_s[:])

                # --- v: DMA-transpose vT -> natural layout + ones cols + mask ---
                nc.sync.dma_start_transpose(out=v_aug[:, :, 0:DH], in_=vT_s[:])
                nc.vector.memset(v_aug[:, :, DH:128], 1.0)
                for t in range(NT):
                    nc.vector.tensor_scalar_mul(v_aug[:, t, :], v_aug[:, t, :],
                                                maskf[:, t:t + 1])
                nc.vector.tensor_copy(vnull_aug[0:1, 0:DH], vn_s[:])
                nc.vector.memset(vnull_aug[0:1, DH:128], 1.0)

                # --- q projection: qT = (Wq*g_in) @ xn^T, ch-split passes ---
                with tc.tile_pool(name="psQ", bufs=8, space="PSUM") as psQ:
                    for ch in range(2):
                        cs = slice(ch * 512, (ch + 1) * 512)
                        pqs = []
                        for mm in range(NT):
                            pq = psQ.tile([128, 512], F32, tag="pq")
                            for t in range(NT):
                                nc.tensor.matmul(
                                    pq[:], wqT_s[:, t, mm * 128:(mm + 1) * 128],
                                    xnT[:, t, cs], start=(t == 0), stop=(t == NT - 1))
                            pqs.append(pq)
                        for mm in range(NT):
                            nc.scalar.copy(qT[:, mm, cs], pqs[mm][:])

                # --- null-token scores for all heads from W_null ---
                with tc.tile_pool(name="psN", bufs=1, space="PSUM") as psN:
                    pnull = psN.tile([H, N], F32, tag="pnull")
                    for ch in range(2):
                        for t in range(NT):
                            nc.tensor.matmul(
                                pnull[:, ch * 512:(ch + 1) * 512], wnT_s[:, t, :],
                                xnT[:, t, ch * 512:(ch + 1) * 512],
                                start=(t == 0), stop=(t == NT - 1))
                    nc.scalar.activation(out=uall[:], in_=pnull[:], func=AF.Exp,
                                         scale=0.125)
                for h in range(H):
                    nc.sync.dma_start(out=unh[0:1, h, :], in_=uall[h:h + 1, :])

            # load woT during phase B (DMA otherwise idle)
            woT_s = poolAB.tile([128, NT, D], F32R, tag="woT")
            nc.sync.dma_start(
                out=woT_s[:], in_=woT_d.ap().rearrange("(t p) c -> p t c", p=128))

            # ---- Phase B: attention (scores + exp + PV + divide) ----
            # visibility: key j visible to query i iff i >= j
            with tc.tile_pool(name="psS", bufs=3, space="PSUM") as psS, \
                 tc.tile_pool(name="psPV", bufs=2, space="PSUM") as psPV, \
                 tc.tile_pool(name="upool", bufs=14) as upool, \
                 tc.tile_pool(name="rpool", bufs=4) as rpool:
                for m in range(NT):          # head pairs (h0=2m, h1=2m+1)
                    utiles = [[None] * NT, [None] * NT]  # [ch][t]
                    for t in range(NT):
                        lo = 128 * t
                        kslice = slice(t * 128, (t + 1) * 128)
                        if t < 4:
                            # ch0 tile: i in [lo, 512)
                            ps0 = psS.tile([128, 2, 512], F32, tag="sc")
                            for ph in range(2):
                                b = 64 * ph
                                nc.tensor.matmul(ps0[:, ph, lo:512],
                                                 kT2[b:b + 64, kslice],
                                                 qT[b:b + 64, m, lo:512],
                                                 start=True, stop=True)
                            u0 = upool.tile([128, 2, 512], BF16, tag="u")
                            nc.scalar.activation(out=u0[:, :, lo:512],
                                                 in_=ps0[:, :, lo:512],
                                                 func=AF.Exp, scale=0.125)
                            # causal diagonal band: zero u above the diagonal
                            for ph in range(2):
                                nc.gpsimd.affine_select(
                                    out=u0[:, ph, lo:lo + 128],
                                    in_=u0[:, ph, lo:lo + 128],
                                    compare_op=ALU.is_ge, fill=0.0,
                                    base=0, pattern=[[1, 128]],
                                    channel_multiplier=-1)
                            utiles[0][t] = u0
                        # ch1 tile: i in [max(lo,512), 1024)
                        l1 = max(lo - 512, 0)
                        ps1 = psS.tile([128, 2, 512], F32, tag="sc")
                        for ph in range(2):
                            b = 64 * ph
                            nc.tensor.matmul(ps1[:, ph, l1:512],
                                             kT2[b:b + 64, kslice],
                                             qT[b:b + 64, m, 512 + l1:1024],
                                             start=True, stop=True)
                        u1 = upool.tile([128, 2, 512], BF16, tag="u")
                        nc.scalar.activation(out=u1[:, :, l1:512],
                                             in_=ps1[:, :, l1:512],
                                             func=AF.Exp, scale=0.125)
                        if t >= 4:
                            for ph in range(2):
                                nc.gpsimd.affine_select(
                                    out=u1[:, ph, l1:l1 + 128],
                                    in_=u1[:, ph, l1:l1 + 128],
                                    compare_op=ALU.is_ge, fill=0.0,
                                    base=0, pattern=[[1, 128]],
                                    channel_multiplier=-1)
                        utiles[1][t] = u1

                        if t == 3:
                            # PV for ch0 can start: all its u tiles exist
                            self_pv(nc, psPV, rpool, utiles, unh, v_aug,
                                    vnull_aug, outT, m, 0)
                    self_pv(nc, psPV, rpool, utiles, unh, v_aug, vnull_aug,
                            outT, m, 1)

        # ============ Phase C: out-projection (natural layout) + LN2 ====
        with tc.tile_pool(name="psC", bufs=3, space="PSUM") as psC, \
             tc.tile_pool(name="poolC", bufs=3) as poolC:
            for it in range(NT):
                i_s = slice(it * 128, (it + 1) * 128)
                po = psC.tile([128, D], F32, tag="po")
                for ch in range(2):
                    for ct in range(NT):
                        nc.tensor.matmul(po[:, ch * 512:(ch + 1) * 512],
                                         outT[:, ct, i_s],
                                         woT_s[:, ct, ch * 512:(ch + 1) * 512],
                                         start=(ct == 0), stop=(ct == NT - 1))
                st = stage.tile([128, 2, 6], F32, tag="bnst")
                nc.vector.bn_stats(out=st[:, 0, :], in_=po[:, 0:512])
                nc.vector.bn_stats(out=st[:, 1, :], in_=po[:, 512:1024])
                mv = stage.tile([128, 2], F32, tag="bnmv")
                nc.vector.bn_aggr(out=mv[:], in_=st[:])
                rstd = stage.tile([128, 1], F32, tag="rstd")
                nc.scalar.activation(out=rstd[:], in_=mv[:, 1:2], func=AF.Sqrt,
                                     bias=eps_t[:], scale=1.0)
                nc.vector.reciprocal(out=rstd[:], in_=rstd[:])
                onorm = poolC.tile([128, D], BF16, tag="onorm")
                nc.vector.tensor_scalar(out=onorm[:], in0=po[:],
                                        scalar1=mv[:, 0:1], scalar2=rstd[:],
                                        op0=ALU.subtract, op1=ALU.mult)
                obf = poolC.tile([128, D], BF16, tag="obf")
                nc.vector.tensor_mul(obf[:], onorm[:], gout_bb[:])
                nc.sync.dma_start(out=out_d[it * 128:(it + 1) * 128, :], in_=obf[:])


def self_pv(nc, psPV, rpool, utiles, unh, v_aug, vnull_aug, outT, m, ch):
    """PV accumulation + null token + denominator divide for both heads of
    pair m, output chunk ch (i in [512ch, 512ch+512))."""
    clo = ch * 512
    for ph in range(2):
        h = 2 * m + ph
        b = 64 * ph
        pv = psPV.tile([128, 512], F32, tag="pv")
        first = True
        for t in range(NT):
            lo = max(128 * t - clo, 0)
            if lo >= 512:
                continue
            nc.tensor.matmul(pv[:, lo:512], v_aug[:, t, :],
                             utiles[ch][t][:, ph, lo:512],
                             start=first, stop=False)
            first = False
        nc.tensor.matmul(pv[:], vnull_aug[0:1, :],
                         unh[0:1, h, clo:clo + 512],
                         start=False, stop=True)
        rinv = rpool.tile([64, 512], F32, tag="rinv")
        nc.vector.reciprocal(out=rinv[:], in_=pv[64:128, :])
        nc.vector.tensor_mul(outT[b:b + 64, m, clo:clo + 512], pv[0:64, :], rinv[:])


_CACHED = None


def _get_nc():
    global _CACHED
    if _CACHED is None:
        nc = bacc.Bacc("TRN2", target_bir_lowering=False, debug=False)
        _emit(nc)
        nc.compile()
        _CACHED = nc
    return _CACHED


def make_in_maps(x, mask, g_in, Wq, Wkv, null_kv, Wo, g_out):
    b = x.shape[0]
    bf = ml_dtypes.bfloat16
    x32 = np.asarray(x, dtype=np.float32)
    xT = np.ascontiguousarray(np.transpose(x32, (0, 2, 1))).astype(bf)
    x_bf = np.ascontiguousarray(x32).astype(bf)
    mask_u8 = np.ascontiguousarray(mask).view(np.uint8) if mask.dtype == np.bool_ \
        else mask.astype(np.uint8)
    g_in = np.asarray(g_in, dtype=np.float32)
    Wq = np.asarray(Wq, dtype=np.float32)
    wq_g = Wq * g_in[None, :]                      # fold g_in into Wq rows' cols
    wqT = np.ascontiguousarray(wq_g.T).astype(bf)  # [D, INNER]
    # W_null[h, :] = null_k . Wq_h rows (g_in folded)  -> [H, D]; stored [D, H]
    nk = np.asarray(null_kv, dtype=np.float32)[0]  # [DH]
    wnull = np.einsum("c,hcd->hd", nk, wq_g.reshape(H, DH, D))
    wnT = np.ascontiguousarray(wnull.T).astype(bf)  # [D, H]
    shared = {
        "wqT": wqT,
        "wnT": wnT,
        "wkvT": np.ascontiguousarray(np.asarray(Wkv, np.float32).T).astype(bf),
        "woT": np.ascontiguousarray(np.asarray(Wo, np.float32).T),
        "vnull": np.ascontiguousarray(np.asarray(null_kv, np.float32)[1:2, :]),
        "gout": np.ascontiguousarray(np.asarray(g_out, np.float32)).astype(bf),
    }
    return [
        {"x": x_bf[c], "xT": xT[c], "mask": mask_u8[c], **shared}
        for c in range(b)
    ]


def kernel(x, mask, g_in, Wq, Wkv, null_kv, Wo, g_out):
    x = np.asarray(x)
    mask = np.asarray(mask)
    b = x.shape[0]
    assert x.shape == (b, N, D) and b == 8
    in_maps = make_in_maps(x, mask, g_in, Wq, Wkv, null_kv, Wo, g_out)
    nc = _get_nc()
    res = run_bass_kernel_spmd(nc, in_maps, core_ids=list(range(b)))
    return np.stack([np.asarray(res.results[c]["out"]).astype(np.float32)
                     for c in range(b)], axis=0)


# revision 36
# speedup vs baseline: 1.2078x; 1.2078x over previous
"""Trainium2 Bass kernel for the masked MQA attention block (nn_Attention_4252017623134).

Sharding: pure data-parallel over batch. b=8 batch elements, 8 NeuronCores,
one batch element per core, weights replicated. No collectives.

Per-core math (n=1024, d=1024, h=16, dh=64, inner=1024):
  context = x                      (pre-norm residual branch feeds K/V)
  xn  = layernorm(x) * g_in        (g_in folded into Wq on host)
  q   = xn @ Wq.T                  (dh^-0.5 folded into exp scale)
  k,v = context @ Wkv.T            (single shared KV head) + null_kv token
  att = softmax(mask(q k^T / 8))   (padding + causal masks)
  out = layernorm(att @ v @ Wo.T) * g_out

v3 design notes:
  * bf16 on all attention matmul operands (full PE rate at any moving-N; the
    cost model charges 1 cycle/moving-row regardless of K/M). Host pre-casts
    x and weights to bf16. Phase C runs f32r x f32r (outT, woT).
  * Zero PE transposes: xT (context for K/V) and xnT both come from
    dma_start_transpose of the x / xn row tiles (16x128 xbar tiles on the
    DMA engines).
  * Fused A/B schedule: the q projection for head pair m+1 is emitted
    between the score and PV matmuls of pair m, so the PE always has dense
    filler while the ACT engine grinds through the exp stream (the phase-B
    pacing engine otherwise).
  * Null-token scores via host-precomposed W_null = null_k @ (Wq*g_in)
    [16 x D]; needs only xnT, so the per-head null rows are ready long
    before the first PV consumes them (SBUF->SBUF DMA broadcast rows).
  * Padding mask: zero masked key rows of v_aug (kills numerator AND
    denominator contributions); exp needs no bias.
  * v_aug columns 64:128 are all-ones, so PV leaves the softmax denominator
    replicated on PSUM partitions 64:128; divide = reciprocal + one fused
    evacuate-multiply per (head, chunk) on DVE.
  * Causal masking: gpsimd affine_select zeroes u above the diagonal on the
    single 128x128 band per (head, j-tile), in SBUF after exp.
"""

import contextlib

import numpy as np
import ml_dtypes

import concourse.bass as bass
import concourse.bacc as bacc
import concourse.tile as tile
import concourse.mybir as mybir
from concourse.bass_utils import run_bass_kernel_spmd
from concourse.masks import make_identity

N = 1024          # sequence length per core
D = 1024          # model dim
H = 16            # query heads
DH = 64           # head dim
INNER = H * DH    # 1024
NT = N // 128     # 8 i-tiles / j-tiles / d-tiles
EPS = 1e-5

F32 = mybir.dt.float32
F32R = mybir.dt.float32r
BF16 = mybir.dt.bfloat16
U8 = mybir.dt.uint8
AF = mybir.ActivationFunctionType
ALU = mybir.AluOpType


DEBUG = False


def _emit(nc):
    x_d = nc.dram_tensor("x", [N, D], BF16, kind="ExternalInput")
    xT_d = nc.dram_tensor("xT", [D, N], BF16, kind="ExternalInput")
    wqT_d = nc.dram_tensor("wqT", [D, INNER], BF16, kind="ExternalInput")
    wnT_d = nc.dram_tensor("wnT", [D, H], BF16, kind="ExternalInput")
    wkvT_d = nc.dram_tensor("wkvT", [D, 2 * DH], BF16, kind="ExternalInput")
    woT_d = nc.dram_tensor("woT", [INNER, D], F32R, kind="ExternalInput")
    vnull_d = nc.dram_tensor("vnull", [1, DH], F32, kind="ExternalInput")
    mask_d = nc.dram_tensor("mask", [N], U8, kind="ExternalInput")
    gout_d = nc.dram_tensor("gout", [D], BF16, kind="ExternalInput")
    out_d = nc.dram_tensor("out", [N, D], BF16, kind="ExternalOutput")
    unh_d = nc.dram_tensor("unh_scratch", [H, N], BF16)

    d_ = dict(x_d=x_d, xT_d=xT_d, wqT_d=wqT_d, wnT_d=wnT_d, wkvT_d=wkvT_d,
              woT_d=woT_d, vnull_d=vnull_d, mask_d=mask_d, gout_d=gout_d,
              out_d=out_d, unh_d=unh_d)
    if DEBUG:
        d_["dbg"] = {
            "kT2": nc.dram_tensor("dbg_kT2", [128, N], BF16, kind="ExternalOutput"),
            "v_aug": nc.dram_tensor("dbg_v_aug", [128, NT, 128], BF16,
                                    kind="ExternalOutput"),
            "qT0": nc.dram_tensor("dbg_qT0", [128, N], BF16, kind="ExternalOutput"),
            "xnT0": nc.dram_tensor("dbg_xnT0", [128, N], BF16,
                                   kind="ExternalOutput"),
            "xT0": nc.dram_tensor("dbg_xT0", [128, N], BF16, kind="ExternalOutput"),
            "uall": nc.dram_tensor("dbg_uall", [H, N], BF16, kind="ExternalOutput"),
            "outT0": nc.dram_tensor("dbg_outT0", [128, N], F32R,
                                    kind="ExternalOutput"),
            "u00": nc.dram_tensor("dbg_u00", [128, 2, 512], BF16,
                                  kind="ExternalOutput"),
            "kv_sb": nc.dram_tensor("dbg_kv_sb", [128, N], BF16,
                                    kind="ExternalOutput"),
            "kv_nat": nc.dram_tensor("dbg_kv_nat", [128, NT, 128], BF16,
                                     kind="ExternalOutput"),
        }
    with tile.TileContext(nc) as tc:
        _emit_tile(nc, tc, d_)
    return nc


def _scores(nc, psS, upool, kT2, qT, m, t):
    """Score matmuls + exp + causal band for head pair m, key tile t.
    Returns (u_ch0 | None, u_ch1)."""
    lo = 128 * t
    ks = slice(t * 128, (t + 1) * 128)
    u0 = None
    if t < 4:
        ps0 = psS.tile([128, 2, 512], F32, tag="sc")
        for ph in range(2):
            b = 64 * ph
            nc.tensor.matmul(ps0[:, ph, lo:512], kT2[b:b + 64, ks],
                             qT[b:b + 64, m, lo:512], start=True, stop=True)
        u0 = upool.tile([128, 2, 512], BF16, tag="u")
        nc.scalar.activation(out=u0[:, :, lo:512], in_=ps0[:, :, lo:512],
                             func=AF.Exp, scale=0.125)
        for ph in range(2):
            nc.gpsimd.affine_select(
                out=u0[:, ph, lo:lo + 128], in_=u0[:, ph, lo:lo + 128],
                compare_op=ALU.is_ge, fill=0.0, base=0,
                pattern=[[1, 128]], channel_multiplier=-1)
    l1 = max(lo - 512, 0)
    ps1 = psS.tile([128, 2, 512], F32, tag="sc")
    for ph in range(2):
        b = 64 * ph
        nc.tensor.matmul(ps1[:, ph, l1:512], kT2[b:b + 64, ks],
                         qT[b:b + 64, m, 512 + l1:1024], start=True, stop=True)
    u1 = upool.tile([128, 2, 512], BF16, tag="u")
    nc.scalar.activation(out=u1[:, :, l1:512], in_=ps1[:, :, l1:512],
                         func=AF.Exp, scale=0.125)
    if t >= 4:
        for ph in range(2):
            nc.gpsimd.affine_select(
                out=u1[:, ph, l1:l1 + 128], in_=u1[:, ph, l1:l1 + 128],
                compare_op=ALU.is_ge, fill=0.0, base=0,
                pattern=[[1, 128]], channel_multiplier=-1)
    return u0, u1


def _pv(nc, psPV, rpool, utiles, unh_pair, v_aug, vnull_aug, outT, m, ch):
    """PV + null token + denominator divide for both heads of pair m,
    output chunk ch (i in [512ch, 512ch+512))."""
    clo = ch * 512
    for ph in range(2):
        b = 64 * ph
        pv = psPV.tile([128, 512], F32, tag="pv")
        first = True
        for t in range(NT):
            lo = max(128 * t - clo, 0)
            if lo >= 512:
                continue
            nc.tensor.matmul(pv[:, lo:512], v_aug[:, t, :],
                             utiles[ch][t][:, ph, lo:512],
                             start=first, stop=False)
            first = False
        nc.tensor.matmul(pv[:], vnull_aug[0:1, :],
                         unh_pair[ph][0:1, clo:clo + 512],
                         start=False, stop=True)
        rinv = rpool.tile([64, 512], F32, tag="rinv")
        nc.vector.reciprocal(out=rinv[:], in_=pv[64:128, :])
        nc.vector.tensor_mul(outT[b:b + 64, m, clo:clo + 512], pv[0:64, :],
                             rinv[:])


def _qproj(nc, psQ, wqT_s, xnT, qT, m, ch):
    """One q-projection chain: qT[:, m, 512ch:512ch+512]."""
    cs = slice(ch * 512, (ch + 1) * 512)
    pq = psQ.tile([128, 512], F32, tag="pq")
    for t in range(NT):
        nc.tensor.matmul(pq[:], wqT_s[:, t, m * 128:(m + 1) * 128],
                         xnT[:, t, cs], start=(t == 0), stop=(t == NT - 1))
    nc.vector.tensor_copy(qT[:, m, cs], pq[:])


def _emit_tile(nc, tc, d_):
    x_d, xT_d = d_["x_d"], d_["xT_d"]
    wqT_d, wnT_d, wkvT_d, woT_d = d_["wqT_d"], d_["wnT_d"], d_["wkvT_d"], d_["woT_d"]
    vnull_d, mask_d = d_["vnull_d"], d_["mask_d"]
    gout_d, out_d, unh_d = d_["gout_d"], d_["out_d"], d_["unh_d"]
    dbg = d_.get("dbg")

    ctx = contextlib.ExitStack()
    with ctx:
        consts = ctx.enter_context(tc.tile_pool(name="consts", bufs=1))
        persist = ctx.enter_context(tc.tile_pool(name="persist", bufs=1))
        stage = ctx.enter_context(tc.tile_pool(name="stage", bufs=4))

        # ---------------- constants ----------------
        eps_t = consts.tile([128, 1], F32)
        nc.vector.memset(eps_t[:], EPS)
        warm = consts.tile([128, 2], F32)
        nc.scalar.activation(out=warm[:, 0:1], in_=eps_t[:], func=AF.Sqrt)
        nc.scalar.activation(out=warm[:, 1:2], in_=eps_t[:], func=AF.Exp)
        maskf = consts.tile([128, NT], F32)
        mask_u8 = consts.tile([128, NT], U8)
        nc.sync.dma_start(out=mask_u8[:],
                          in_=bass.AP(tensor=mask_d, offset=0,
                                      ap=[[1, 128], [128, NT]]))
        nc.vector.tensor_copy(maskf[:], mask_u8[:])
        gout_bb = consts.tile([128, D], BF16)
        nc.sync.dma_start(out=gout_bb[:],
                          in_=bass.AP(tensor=gout_d, offset=0,
                                      ap=[[0, 128], [1, D]]))
        ident = consts.tile([128, 128], BF16)
        make_identity(nc, ident[:])

        # ------------- whole-kernel persistent tiles -------------
        kT2 = persist.tile([128, N], BF16, tag="kT2")        # both 64-halves = k^T
        v_aug = persist.tile([128, NT, 128], BF16, tag="v_aug")  # [:,t,0:64]=v, 64:=ones
        vnull_aug = persist.tile([1, 128], BF16, tag="vnull")    # [v_null | ones]
        outT = persist.tile([128, NT, N], F32R, tag="outT")  # attn out^T, divided

        with tc.tile_pool(name="poolAB", bufs=1) as poolAB:
            qT = poolAB.tile([128, NT, N], BF16, tag="qT")
            xnT = poolAB.tile([128, NT, N], BF16, tag="xnT")
            wqT_s = poolAB.tile([128, NT, INNER], BF16, tag="wqT")
            wkvT_s = poolAB.tile([128, NT, 2 * DH], BF16, tag="wkvT")
            wnT_s = poolAB.tile([128, NT, H], BF16, tag="wnT")
            woT_s = poolAB.tile([128, NT, D], F32R, tag="woT")

            with tc.tile_pool(name="poolA", bufs=1) as poolA:
                vT_s = poolA.tile([64, N], BF16, tag="vT_s")
                uall = poolA.tile([H, N], BF16, tag="uall")
                x_tiles = []
                xT_tiles = []

                # weight + x/xT loads (wkv first: kv matmuls are earliest PE work)
                nc.sync.dma_start(
                    out=wkvT_s[:],
                    in_=wkvT_d.ap().rearrange("(t p) c -> p t c", p=128))
                vn_s = stage.tile([1, DH], F32, tag="nk2")
                nc.sync.dma_start(out=vn_s[:], in_=vnull_d.ap())
                for it in range(NT):
                    xs = poolA.tile([128, D], BF16, tag=f"x{it}")
                    nc.sync.dma_start(out=xs[:], in_=x_d[it * 128:(it + 1) * 128, :])
                    x_tiles.append(xs)
                    xt = poolA.tile([128, N], BF16, tag=f"xT{it}")
                    nc.sync.dma_start(out=xt[:],
                                      in_=xT_d[it * 128:(it + 1) * 128, :])
                    xT_tiles.append(xt)
                nc.sync.dma_start(
                    out=wqT_s[:],
                    in_=wqT_d.ap().rearrange("(t p) c -> p t c", p=128))
                nc.sync.dma_start(
                    out=wnT_s[:],
                    in_=wnT_d.ap().rearrange("(t p) c -> p t c", p=128))

                # ---- LN1 + PE transposes (normed -> xnT) + kv chains ----
                with tc.tile_pool(name="psKV", bufs=1, space="PSUM") as psKV, \
                     tc.tile_pool(name="psT", bufs=2, space="PSUM") as psT, \
                     tc.tile_pool(name="psN", bufs=1, space="PSUM") as psN:
                    pkv = psKV.tile([128, N], F32, tag="pkv")
                    for it in range(NT):
                        x_s = x_tiles[it]
                        ib = slice(it * 128, (it + 1) * 128)
                        st = stage.tile([128, 2, 6], F32, tag="bnst")
                        nc.vector.bn_stats(out=st[:, 0, :], in_=x_s[:, 0:512])
                        nc.vector.bn_stats(out=st[:, 1, :], in_=x_s[:, 512:1024])
                        mv = stage.tile([128, 2], F32, tag="bnmv")
                        nc.vector.bn_aggr(out=mv[:], in_=st[:])
                        rstd = stage.tile([128, 1], F32, tag="rstd")
                        nc.scalar.activation(out=rstd[:], in_=mv[:, 1:2],
                                             func=AF.Sqrt, bias=eps_t[:], scale=1.0)
                        nc.vector.reciprocal(out=rstd[:], in_=rstd[:])
                        xn_s = stage.tile([128, D], BF16, tag="xnft")
                        nc.vector.tensor_scalar(out=xn_s[:], in0=x_s[:],
                                                scalar1=mv[:, 0:1], scalar2=rstd[:],
                                                op0=ALU.subtract, op1=ALU.mult)
                        # PE transpose xn -> xnT columns for this i-block
                        pt = psT.tile([128, NT, 128], BF16, tag="pt")
                        for t in range(NT):
                            nc.tensor.transpose(pt[:, t, :],
                                                xn_s[:, t * 128:(t + 1) * 128],
                                                ident[:])
                        nc.vector.tensor_copy(xnT[:, :, ib], pt[:])
                        # kv accumulation chain for this d-tile (rhs = xT row)
                        for ch in range(2):
                            nc.tensor.matmul(pkv[:, ch * 512:(ch + 1) * 512],
                                             wkvT_s[:, it, :],
                                             xT_tiles[it][:, ch * 512:(ch + 1) * 512],
                                             start=(it == 0), stop=(it == NT - 1))
                    # evacuate kv: k duplicated into both kT2 halves, v -> vT_s
                    nc.scalar.copy(kT2[0:64, :], pkv[0:64, :])
                    nc.scalar.copy(kT2[64:128, :], pkv[0:64, :])
                    nc.vector.tensor_copy(vT_s[:], pkv[64:128, :])
                    # v natural via PE transposes
                    pv_t = psT.tile([128, NT, DH], BF16, tag="pvt")
                    for t in range(NT):
                        nc.tensor.transpose(pv_t[:, t, :],
                                            vT_s[:, t * 128:(t + 1) * 128],
                                            ident[0:64, 0:64])
                    nc.vector.tensor_copy(v_aug[:, :, 0:DH], pv_t[:])
                    nc.gpsimd.memset(v_aug[:, :, DH:128], 1.0)
                    for t in range(NT):
                        nc.gpsimd.tensor_scalar_mul(v_aug[:, t, :], v_aug[:, t, :],
                                                    maskf[:, t:t + 1])
                    nc.vector.tensor_copy(vnull_aug[0:1, 0:DH], vn_s[:])
                    nc.vector.memset(vnull_aug[0:1, DH:128], 1.0)

                    # null-token scores for all heads from W_null (only needs xnT)
                    pnull = psN.tile([H, N], F32, tag="pnull")
                    for ch in range(2):
                        for t in range(NT):
                            nc.tensor.matmul(
                                pnull[:, ch * 512:(ch + 1) * 512], wnT_s[:, t, :],
                                xnT[:, t, ch * 512:(ch + 1) * 512],
                                start=(t == 0), stop=(t == NT - 1))
                    nc.scalar.activation(out=uall[:], in_=pnull[:], func=AF.Exp,
                                         scale=0.125)
                    nc.sync.dma_start(out=unh_d.ap(), in_=uall[:])
                    if dbg:
                        nc.sync.dma_start(out=dbg["kT2"].ap(), in_=kT2[:])
                        nc.sync.dma_start(out=dbg["v_aug"].ap(), in_=v_aug[:])
                        nc.sync.dma_start(out=dbg["xnT0"].ap(), in_=xnT[:, 0, :])
                        nc.sync.dma_start(out=dbg["uall"].ap(), in_=uall[:])

            # woT load for phase C (DMA idle during B)
            nc.sync.dma_start(
                out=woT_s[:], in_=woT_d.ap().rearrange("(t p) c -> p t c", p=128))

            # ---- fused q-proj + attention loop over head pairs ----
            with tc.tile_pool(name="psQ", bufs=2, space="PSUM") as psQ, \
                 tc.tile_pool(name="psS", bufs=2, space="PSUM") as psS, \
                 tc.tile_pool(name="psPV", bufs=2, space="PSUM") as psPV, \
                 tc.tile_pool(name="upool", bufs=14) as upool, \
                 tc.tile_pool(name="rpool", bufs=4) as rpool, \
                 tc.tile_pool(name="unhp", bufs=4) as unhp:
                _qproj(nc, psQ, wqT_s, xnT, qT, 0, 0)
                _qproj(nc, psQ, wqT_s, xnT, qT, 0, 1)
                unh_next = None
                for m in range(NT):
                    # null-exp rows for this pair (loaded at previous pair)
                    if unh_next is None:
                        unh_next = []
                        for ph in range(2):
                            ut = unhp.tile([1, N], BF16, tag="unh")
                            nc.sync.dma_start(out=ut[:],
                                              in_=unh_d[2 * m + ph:2 * m + ph + 1, :])
                            unh_next.append(ut)
                    unh_pair = unh_next
                    utiles = [[None] * NT, [None] * NT]
                    # scores t=0..3 (both chunks)
                    for t in range(4):
                        utiles[0][t], utiles[1][t] = _scores(
                            nc, psS, upool, kT2, qT, m, t)
                    # filler: next pair's q ch0 + unh prefetch
                    if m + 1 < NT:
                        unh_next = []
                        for ph in range(2):
                            ut = unhp.tile([1, N], BF16, tag="unh")
                            nc.sync.dma_start(
                                out=ut[:],
                                in_=unh_d[2 * m + 2 + ph:2 * m + 3 + ph, :])
                            unh_next.append(ut)
                        _qproj(nc, psQ, wqT_s, xnT, qT, m + 1, 0)
                    # scores t=4..7 (ch1 only)
                    for t in range(4, NT):
                        _, utiles[1][t] = _scores(nc, psS, upool, kT2, qT, m, t)
                    # PV ch0 (needs exps t0..3 - ACT had the q-chain to catch up)
                    _pv(nc, psPV, rpool, utiles, unh_pair, v_aug, vnull_aug,
                        outT, m, 0)
                    if m + 1 < NT:
                        _qproj(nc, psQ, wqT_s, xnT, qT, m + 1, 1)
                    _pv(nc, psPV, rpool, utiles, unh_pair, v_aug, vnull_aug,
                        outT, m, 1)
                    if dbg and m == 0:
                        nc.sync.dma_start(out=dbg["qT0"].ap(), in_=qT[:, 0, :])
                        nc.sync.dma_start(out=dbg["outT0"].ap(), in_=outT[:, 0, :])
                        nc.sync.dma_start(out=dbg["u00"].ap(), in_=utiles[0][0][:])

        # ============ Phase C: out-projection (natural layout) + LN2 ====
        with tc.tile_pool(name="psC", bufs=3, space="PSUM") as psC, \
             tc.tile_pool(name="poolC", bufs=3) as poolC:
            for it in range(NT):
                i_s = slice(it * 128, (it + 1) * 128)
                po = psC.tile([128, D], F32, tag="po")
                for ch in range(2):
                    for ct in range(NT):
                        nc.tensor.matmul(po[:, ch * 512:(ch + 1) * 512],
                                         outT[:, ct, i_s],
                                         woT_s[:, ct, ch * 512:(ch + 1) * 512],
                                         start=(ct == 0), stop=(ct == NT - 1))
                st = stage.tile([128, 2, 6], F32, tag="bnst")
                nc.vector.bn_stats(out=st[:, 0, :], in_=po[:, 0:512])
                nc.vector.bn_stats(out=st[:, 1, :], in_=po[:, 512:1024])
                mv = stage.tile([128, 2], F32, tag="bnmv")
                nc.vector.bn_aggr(out=mv[:], in_=st[:])
                rstd = stage.tile([128, 1], F32, tag="rstd")
                nc.scalar.activation(out=rstd[:], in_=mv[:, 1:2], func=AF.Sqrt,
                                     bias=eps_t[:], scale=1.0)
                nc.vector.reciprocal(out=rstd[:], in_=rstd[:])
                onorm = poolC.tile([128, D], BF16, tag="onorm")
                nc.vector.tensor_scalar(out=onorm[:], in0=po[:],
                                        scalar1=mv[:, 0:1], scalar2=rstd[:],
                                        op0=ALU.subtract, op1=ALU.mult)
                obf = poolC.tile([128, D], BF16, tag="obf")
                nc.gpsimd.tensor_mul(obf[:], onorm[:], gout_bb[:])
                nc.sync.dma_start(out=out_d[it * 128:(it + 1) * 128, :], in_=obf[:])


_CACHED = None


def _get_nc():
    global _CACHED
    if _CACHED is None:
        nc = bacc.Bacc("TRN2", target_bir_lowering=False, debug=False)
        _emit(nc)
        nc.compile()
        _CACHED = nc
    return _CACHED


def make_in_maps(x, mask, g_in, Wq, Wkv, null_kv, Wo, g_out):
    b = x.shape[0]
    bf = ml_dtypes.bfloat16
    x32 = np.asarray(x, dtype=np.float32)
    x_bf = np.ascontiguousarray(x32).astype(bf)
    xT_bf = np.ascontiguousarray(np.transpose(x32, (0, 2, 1))).astype(bf)
    mask_u8 = np.ascontiguousarray(mask).view(np.uint8) if mask.dtype == np.bool_ \
        else mask.astype(np.uint8)
    g_in = np.asarray(g_in, dtype=np.float32)
    Wq = np.asarray(Wq, dtype=np.float32)
    wq_g = Wq * g_in[None, :]                      # fold g_in into Wq
    wqT = np.ascontiguousarray(wq_g.T).astype(bf)  # [D, INNER]
    # W_null[h, :] = null_k . Wq_h rows (g_in folded)  -> stored [D, H]
    nk = np.asarray(null_kv, dtype=np.float32)[0]  # [DH]
    wnull = np.einsum("c,hcd->hd", nk, wq_g.reshape(H, DH, D))
    wnT = np.ascontiguousarray(wnull.T).astype(bf)  # [D, H]
    shared = {
        "wqT": wqT,
        "wnT": wnT,
        "wkvT": np.ascontiguousarray(np.asarray(Wkv, np.float32).T).astype(bf),
        "woT": np.ascontiguousarray(np.asarray(Wo, np.float32).T),
        "vnull": np.ascontiguousarray(np.asarray(null_kv, np.float32)[1:2, :]),
        "gout": np.ascontiguousarray(np.asarray(g_out, np.float32)).astype(bf),
    }
    return [
        {"x": x_bf[c], "xT": xT_bf[c], "mask": mask_u8[c], **shared}
        for c in range(b)
    ]


def kernel(x, mask, g_in, Wq, Wkv, null_kv, Wo, g_out):
    x = np.asarray(x)
    mask = np.asarray(mask)
    b = x.shape[0]
    assert x.shape == (b, N, D) and b == 8
    in_maps = make_in_maps(x, mask, g_in, Wq, Wkv, null_kv, Wo, g_out)
    nc = _get_nc()
    res = run_bass_kernel_spmd(nc, in_maps, core_ids=list(range(b)))
    return np.stack([np.asarray(res.results[c]["out"]).astype(np.float32)
                     for c in range(b)], axis=0)
